# revision 27
# baseline (speedup 1.0000x reference)
"""Trainium2 Bass kernel for nn_LocalFeatue (PPF local feature module).

Shards the N (center) axis x batch across 8 NeuronCores: core c handles
batch c//4, centers [(c%4)*1024, (c%4+1)*1024).  Ball query, gather, PPF
features, 3x (1x1 conv + GroupNorm) and K-maxpool all run on device; the
GroupNorm statistics are allreduced across cores (tiny 2x16 f32 tensors).

Per-core pair list (per GPSIMD core q, one tile of 128 centers each):
list position i = 512*t + 16*s + c16 maps to (tile t, slot s, center
16*q + c16 of tile t).  Pair-plane mapping: partition p = i % 128,
plane column f = 32*q + i // 128.  Conv column = 4096*q + i.
Output column = 128*q + 16*t + c16 -> center 128*t + 16*q + c16
(host permutes).

SBUF tiles pad to 4KB/partition, so small constants and scratch are
packed into a few column-sliced tiles (cst/cstw/cstm/statbuf/scales).
"""

import sys
sys.path.insert(0, '/opt/trn_rl_repo')

import numpy as np
import concourse.bacc as bacc
import concourse.bass as bass
import concourse.mybir as mybir
import concourse.tile as tile
import concourse.bass_utils as bass_utils

dt = mybir.dt
alu = mybir.AluOpType
AF = mybir.ActivationFunctionType
AX = mybir.AxisListType

B, N, K = 2, 4096, 32
R2 = np.float32(0.1 * 0.1)
NCEN = 1024
TILES = 8
P = 128
PAIRS = NCEN * K
EPS = 1e-5
CNT01 = float(8 * K * N)
CNT2 = float(16 * K * N)
PI = float(np.pi)
F16 = dt.float16

# cst (f32, 128 x 320) column map
C_NEG1 = 0
C_GW1 = 1       # (64, 8)
C_GW2 = 9       # (64, 8)
C_GI1 = 17      # (64, 8)
C_GI2 = 25      # (128, 8)
C_GE8 = 33      # (8, 64)
C_GE16 = 97     # (8, 128)
C_A0T = 225     # (112, 32)
C_G0, C_B0, C_G1, C_B1, C_G2, C_B2 = 257, 258, 259, 260, 261, 262
C_BSELC = 263   # (2, 1)
C_BS0 = 264     # (16, 1)
C_BS1 = 265     # (16, 1)
CST_W = 320

# cstm (f32, 128 x 128) column map (mid-phase consts)
M_THR = 0       # (128, 8)
M_IOTA32 = 8    # (128, 32)
M_ID = 40       # (128, 64) 2-stacked 64x64 identity
M_WA, M_WB, M_WC = 104, 105, 106
M_ID128 = 128   # (128, 128) identity
CSTM_W = 256

# statbuf (f32, 128 x 128) column map (kernel-written smalls)
S_SH1 = 0
S_SH2 = 1
S_Q1CH = 2
S_Q2CH = 3
S_Q1P = 4       # (64, 16)
S_Q2P = 20      # (128, 32)
S_MSB = 52      # (14, 14)
S_LAY = 66      # per layer L: 12 cols at 66+12L: mean8,ex2,m2,var8,sd8,rsd8,
                #   t0q,t1q,t0s,t1s,sqq,sqs
S_MFLAT = 102   # (112, 2)
S_AR2 = 104     # (2, 16)
S_SROW = 120    # per layer L: q at 120+2L, s at 121+2L
S_TMP = 126     # 126+layer (64/128,1) tmp
STAT_W = 136

# scales (f32, 128 x 16): per layer L: rsdC 4L, meanC 4L+1, scale 4L+2, shift 4L+3


def build_program(skip_collective=False):
    nc = bacc.Bacc("TRN2", target_bir_lowering=False, debug=False,
                   enable_asserts=False, num_devices=8)
    f32 = dt.float32

    def din(name, shape, dtype=f32):
        return nc.dram_tensor(name, shape, dtype, kind="ExternalInput")

    cen4T_d = din("cen4T", [4, NCEN])
    pts4_d = din("pts4", [4, N])
    src16_d = din("src16", [P, N])
    cpl_d = din("cpl", [P, 6 * 256])
    cst_d = din("cst", [P, CST_W])
    cstm_d = din("cstm", [P, CSTM_W])
    cstw_d = din("cstw", [P, 256], F16)

    outx_d = nc.dram_tensor("outx", [128, NCEN], f32, kind="ExternalOutput")
    outr_d = nc.dram_tensor("outr", [128, NCEN], f32, kind="ExternalOutput")

    with tile.TileContext(nc) as tc:
        with tc.tile_pool(name="per", bufs=1) as per, \
             tc.tile_pool(name="midout", bufs=1) as midout, \
             tc.tile_pool(name="dram", bufs=1, space="DRAM") as dram, \
             tc.tile_pool(name="psmall", bufs=2, space="PSUM") as psmall:

            cst = per.tile([P, CST_W], f32, tag="cst")
            nc.sync.dma_start(cst[:], cst_d.ap())
            cstw = per.tile([P, 256], F16, tag="cstw")
            nc.sync.dma_start(cstw[:], cstw_d.ap())
            statbuf = per.tile([P, STAT_W], f32, tag="statbuf")
            scales = per.tile([P, 16], f32, tag="scales")
            outmax = per.tile([P, NCEN], f32, tag="outmax")
            outx = per.tile([P, NCEN], f32, tag="outx")
            outr = per.tile([P, NCEN], f32, tag="outr")

            def cc(col, w=1, rows=128, r0=0):
                return cst[r0:r0 + rows, col:col + w]

            w0T = cstw[0:14, 0:64]
            w1T = cstw[0:64, 64:128]
            w2T = cstw[0:64, 128:256]

            idxw = midout.tile([P, 256], dt.int16, tag="idxw")
            cstm = midout.tile([P, CSTM_W], f32, tag="cstm")
            nc.sync.dma_start(cstm[:], cstm_d.ap())

            # ---------- helpers ----------
            def allreduce_stats(qt, st, layer):
                """qt, st: (8,1) APs of partial sums -> [q8, s8] (8,1) APs."""
                L0 = S_LAY + 12 * layer
                t0q = statbuf[0:8, L0 + 6:L0 + 7]
                t1q = statbuf[0:8, L0 + 7:L0 + 8]
                t0s = statbuf[0:8, L0 + 8:L0 + 9]
                t1s = statbuf[0:8, L0 + 9:L0 + 10]
                bs0 = cc(C_BS0, rows=8)
                bs1 = cc(C_BS1, rows=8)
                nc.vector.tensor_scalar(out=t0q, in0=qt, scalar1=bs0, scalar2=None, op0=alu.mult)
                nc.vector.tensor_scalar(out=t1q, in0=qt, scalar1=bs1, scalar2=None, op0=alu.mult)
                nc.vector.tensor_scalar(out=t0s, in0=st, scalar1=bs0, scalar2=None, op0=alu.mult)
                nc.vector.tensor_scalar(out=t1s, in0=st, scalar1=bs1, scalar2=None, op0=alu.mult)
                arin = dram.tile([2, 16], f32, tag=f"arin{layer}", name=f"arin{layer}")
                arout = dram.tile([2, 16], f32, tag=f"arout{layer}", name=f"arout{layer}")
                nc.sync.dma_start(arin[0:1, 0:8], t0q)
                nc.sync.dma_start(arin[1:2, 0:8], t1q)
                nc.sync.dma_start(arin[0:1, 8:16], t0s)
                nc.sync.dma_start(arin[1:2, 8:16], t1s)
                if skip_collective:
                    nc.sync.dma_start(arout[:], arin[:])
                else:
                    nc.gpsimd.collective_compute(
                        "AllReduce", alu.add, replica_groups=[list(range(8))],
                        ins=[arin[:]], outs=[arout[:]])
                ar2 = statbuf[0:2, S_AR2:S_AR2 + 16]
                nc.sync.dma_start(ar2, arout[:])
                outs = []
                for half in (0, 1):
                    psr = psmall.tile([8, 1], f32, tag="ps", name=f"psr{layer}_{half}")
                    nc.tensor.matmul(psr[:], ar2[:, 8 * half:8 * half + 8],
                                     cc(C_BSELC, rows=2), start=True, stop=True)
                    srow = statbuf[0:8, S_SROW + 2 * layer + half:S_SROW + 2 * layer + half + 1]
                    nc.scalar.copy(srow, psr[:])
                    outs.append(srow)
                return outs

            def gn_affine(q8, s8, cnt, gcol, bcol, gecol, gerows, C, layer):
                L0 = S_LAY + 12 * layer
                mean8 = statbuf[0:8, L0 + 0:L0 + 1]
                ex2 = statbuf[0:8, L0 + 1:L0 + 2]
                m2 = statbuf[0:8, L0 + 2:L0 + 3]
                var8 = statbuf[0:8, L0 + 3:L0 + 4]
                sd8 = statbuf[0:8, L0 + 4:L0 + 5]
                rsd8 = statbuf[0:8, L0 + 5:L0 + 6]
                nc.vector.tensor_scalar(out=mean8, in0=s8, scalar1=1.0 / cnt,
                                        scalar2=None, op0=alu.mult)
                nc.vector.tensor_scalar(out=ex2, in0=q8, scalar1=1.0 / cnt,
                                        scalar2=None, op0=alu.mult)
                nc.vector.tensor_tensor(out=m2, in0=mean8, in1=mean8, op=alu.mult)
                nc.vector.tensor_tensor(out=var8, in0=ex2, in1=m2, op=alu.subtract)
                nc.vector.tensor_scalar(out=var8, in0=var8, scalar1=EPS,
                                        scalar2=None, op0=alu.add)
                nc.scalar.activation(sd8, var8, AF.Sqrt)
                nc.vector.reciprocal(rsd8, sd8)
                gexp = cst[0:8, gecol:gecol + C]
                rsdC = scales[0:C, 4 * layer:4 * layer + 1]
                meanC = scales[0:C, 4 * layer + 1:4 * layer + 2]
                pex = psmall.tile([C, 1], f32, tag="ps2", name=f"pex{layer}")
                nc.tensor.matmul(pex[:], gexp, rsd8, start=True, stop=True)
                nc.scalar.copy(rsdC, pex[:])
                pex2 = psmall.tile([C, 1], f32, tag="ps2", name=f"pex2{layer}")
                nc.tensor.matmul(pex2[:], gexp, mean8, start=True, stop=True)
                nc.scalar.copy(meanC, pex2[:])
                scale = scales[0:C, 4 * layer + 2:4 * layer + 3]
                shift = scales[0:C, 4 * layer + 3:4 * layer + 4]
                tmp = statbuf[0:C, S_TMP + layer:S_TMP + layer + 1]
                nc.vector.tensor_tensor(out=scale, in0=cc(gcol, rows=C), in1=rsdC, op=alu.mult)
                nc.vector.tensor_tensor(out=tmp, in0=meanC, in1=scale, op=alu.mult)
                nc.vector.tensor_tensor(out=shift, in0=cc(bcol, rows=C), in1=tmp, op=alu.subtract)
                return scale, shift

            with tc.tile_pool(name="bigA", bufs=1) as bigA:
                # ================= ball-query era =================
                with tc.tile_pool(name="midA", bufs=1) as midA:
                    iota16 = midA.tile([P, N], dt.int16, tag="iota16")
                    nc.gpsimd.iota(iota16[:], pattern=[[1, N]], base=0,
                                   channel_multiplier=0)
                    idx48 = midA.tile([P, 48 * TILES], dt.int16, tag="idx48")
                    cnts = midA.tile([P, TILES], f32, tag="cnts")
                    cen4T = midA.tile([4, NCEN], f32, tag="cen4T")
                    nc.sync.dma_start(cen4T[:], cen4T_d.ap())
                    pts4 = midA.tile([4, N], f32, tag="pts4")
                    nc.sync.dma_start(pts4[:], pts4_d.ap())

                    with tc.tile_pool(name="ball", bufs=2) as ball, \
                         tc.tile_pool(name="pdist", bufs=2, space="PSUM") as pdist:
                        for t in range(TILES):
                            mask = ball.tile([P, N], dt.bfloat16, tag="mask")
                            for h in range(4):
                                pd = pdist.tile([P, 1024], f32, tag="pd", name="pd")
                                for j in range(2):
                                    c0 = 1024 * h + 512 * j
                                    nc.tensor.matmul(
                                        pd[:, 512 * j:512 * (j + 1)],
                                        cen4T[:, 128 * t:128 * (t + 1)],
                                        pts4[:, c0:c0 + 512],
                                        start=True, stop=True)
                                nc.vector.tensor_scalar(
                                    out=mask[:, 1024 * h:1024 * (h + 1)],
                                    in0=pd[:], scalar1=cstm[:, M_THR + t:M_THR + t + 1],
                                    scalar2=None, op0=alu.is_le)
                            rank = ball.tile([P, N], f32, tag="rank")
                            nc.vector.tensor_tensor_scan(
                                out=rank[:], data0=mask[:], data1=mask[:],
                                initial=0.0, op0=alu.add, op1=alu.max)
                            nc.scalar.copy(cnts[:, t:t + 1], rank[:, N - 1:N])
                            sel = ball.tile([P, N], dt.bfloat16, tag="sel")
                            nc.vector.scalar_tensor_tensor(
                                out=sel[:], in0=rank[:], scalar=47.0, in1=mask[:],
                                op0=alu.min, op1=alu.mult)
                            slot16 = ball.tile([P, N], dt.int16, tag="slot16")
                            nc.scalar.activation(slot16[:], sel[:], AF.Identity,
                                                 bias=cc(C_NEG1))
                            nc.gpsimd.local_scatter(
                                idx48[:, 48 * t:48 * t + 48], iota16[:], slot16[:],
                                channels=128, num_elems=48, num_idxs=N)
                            # pad-fix
                            bsc = ball.tile([P, 160], f32, tag="bsc")
                            pmf = bsc[:, 0:32]
                            firstf = bsc[:, 32:33]
                            idxf = bsc[:, 48:80]
                            dtl = bsc[:, 80:112]
                            etl = bsc[:, 112:144]
                            nc.vector.tensor_scalar(
                                out=pmf, in0=cstm[:, M_IOTA32:M_IOTA32 + 32],
                                scalar1=cnts[:, t:t + 1], scalar2=None, op0=alu.is_lt)
                            nc.scalar.copy(firstf, idx48[:, 48 * t:48 * t + 1])
                            nc.scalar.copy(idxf, idx48[:, 48 * t:48 * t + 32])
                            nc.vector.tensor_scalar(
                                out=dtl, in0=idxf, scalar1=firstf,
                                scalar2=None, op0=alu.subtract)
                            nc.vector.tensor_tensor(out=etl, in0=dtl, in1=pmf,
                                                    op=alu.mult)
                            nc.vector.tensor_scalar(
                                out=idxw[:, 32 * t:32 * t + 32], in0=etl,
                                scalar1=firstf, scalar2=None, op0=alu.add)

                # ================= gather/feature era =================
                with tc.tile_pool(name="midB", bufs=1) as midB:
                    src16 = midB.tile([P, N], f32, tag="m2", name="src16")
                    nc.sync.dma_start(src16[:], src16_d.ap())
                    gath = midB.tile([P, N], f32, tag="mbig", name="gath")
                    nc.gpsimd.ap_gather(
                        gath[:].rearrange("p (n d) -> p n d", d=1),
                        src16[:].rearrange("p (n d) -> p n d", d=1),
                        idxw[:], channels=128, num_elems=N, d=1, num_idxs=4096)

                    gpl = midB.tile([P, 6 * 256], f32, tag="gpl")
                    with tc.tile_pool(name="ptr", bufs=2, space="PSUM") as ptr:
                        for Q in range(2):
                            for half in range(2):
                                ptile = ptr.tile([P, 1024], f32, tag="ptile",
                                                 name="ptile")
                                for jj in range(16):
                                    j = 16 * half + jj
                                    nc.tensor.transpose(
                                        ptile[:, 64 * jj:64 * jj + 64],
                                        gath[64 * Q:64 * Q + 64,
                                             128 * j:128 * j + 128],
                                        cstm[64 * Q:64 * Q + 64, M_ID:M_ID + 64])
                                for sub in range(4):
                                    g = 4 * Q + sub
                                    src_ap = ptile[:].rearrange(
                                        "p (j r) -> p j r", j=16)[:, :, 16 * sub:16 * sub + 6]
                                    dst_ap = gpl[:].rearrange(
                                        "p (r g j) -> p g j r", r=6, g=8)[
                                        :, g, 16 * half:16 * half + 16, :]
                                    nc.vector.tensor_copy(dst_ap, src_ap)

                    cpl = midB.tile([P, 6 * 256], f32, tag="cpl")
                    nc.sync.dma_start(cpl[:], cpl_d.ap())
                    fsc = midB.tile([P, 17 * 256], f32, tag="fsc")
                    fpl = midB.tile([P, 14 * 256], f32, tag="fpl")

                    def gp(r):
                        return gpl[:, 256 * r:256 * (r + 1)]

                    def cp(r):
                        return cpl[:, 256 * r:256 * (r + 1)]

                    def fp(r):
                        return fpl[:, 256 * r:256 * (r + 1)]

                    def sc(r):
                        return fsc[:, 256 * r:256 * (r + 1)]

                    wa = cstm[:, M_WA:M_WA + 1]
                    wb = cstm[:, M_WB:M_WB + 1]
                    wc = cstm[:, M_WC:M_WC + 1]
                    for r in range(3):
                        nc.vector.tensor_scalar(out=fp(r), in0=cp(r),
                                                scalar1=wa, scalar2=None, op0=alu.mult)
                        nc.vector.tensor_scalar(out=fp(3 + r), in0=gp(r),
                                                scalar1=wa, scalar2=None, op0=alu.mult)
                    rel = [sc(0), sc(1), sc(2)]
                    for r in range(3):
                        nc.vector.tensor_tensor(out=rel[r], in0=gp(r), in1=cp(r),
                                                op=alu.subtract)
                        nc.vector.tensor_scalar(out=fp(6 + r), in0=rel[r],
                                                scalar1=wb, scalar2=None, op0=alu.mult)
                    d2, t2, dist = sc(3), sc(4), sc(5)
                    nc.vector.tensor_tensor(out=d2, in0=rel[0], in1=rel[0], op=alu.mult)
                    nc.vector.tensor_tensor(out=t2, in0=rel[1], in1=rel[1], op=alu.mult)
                    nc.vector.tensor_tensor(out=d2, in0=d2, in1=t2, op=alu.add)
                    nc.vector.tensor_tensor(out=t2, in0=rel[2], in1=rel[2], op=alu.mult)
                    nc.vector.tensor_tensor(out=d2, in0=d2, in1=t2, op=alu.add)
                    nc.scalar.activation(dist, d2, AF.Sqrt)
                    nc.vector.tensor_scalar(out=fp(9), in0=dist, scalar1=wb,
                                            scalar2=None, op0=alu.mult)

                    cx, cy, cz = sc(6), sc(7), sc(8)
                    u, v = sc(9), sc(10)
                    y2, ynorm, xdot = sc(11), sc(12), sc(13)
                    deg, rec, at = sc(14), sc(15), sc(16)

                    def emit_angle(out_ap, a, b):
                        nc.vector.tensor_tensor(out=u, in0=a[1], in1=b[2], op=alu.mult)
                        nc.vector.tensor_tensor(out=v, in0=a[2], in1=b[1], op=alu.mult)
                        nc.vector.tensor_tensor(out=cx, in0=u, in1=v, op=alu.subtract)
                        nc.vector.tensor_tensor(out=u, in0=a[2], in1=b[0], op=alu.mult)
                        nc.vector.tensor_tensor(out=v, in0=a[0], in1=b[2], op=alu.mult)
                        nc.vector.tensor_tensor(out=cy, in0=u, in1=v, op=alu.subtract)
                        nc.vector.tensor_tensor(out=u, in0=a[0], in1=b[1], op=alu.mult)
                        nc.vector.tensor_tensor(out=v, in0=a[1], in1=b[0], op=alu.mult)
                        nc.vector.tensor_tensor(out=cz, in0=u, in1=v, op=alu.subtract)
                        nc.vector.tensor_tensor(out=y2, in0=cx, in1=cx, op=alu.mult)
                        nc.vector.tensor_tensor(out=u, in0=cy, in1=cy, op=alu.mult)
                        nc.vector.tensor_tensor(out=y2, in0=y2, in1=u, op=alu.add)
                        nc.vector.tensor_tensor(out=u, in0=cz, in1=cz, op=alu.mult)
                        nc.vector.tensor_tensor(out=y2, in0=y2, in1=u, op=alu.add)
                        nc.scalar.activation(ynorm, y2, AF.Sqrt)
                        nc.vector.tensor_tensor(out=xdot, in0=a[0], in1=b[0], op=alu.mult)
                        nc.vector.tensor_tensor(out=u, in0=a[1], in1=b[1], op=alu.mult)
                        nc.vector.tensor_tensor(out=xdot, in0=xdot, in1=u, op=alu.add)
                        nc.vector.tensor_tensor(out=u, in0=a[2], in1=b[2], op=alu.mult)
                        nc.vector.tensor_tensor(out=xdot, in0=xdot, in1=u, op=alu.add)
                        nc.vector.scalar_tensor_tensor(
                            out=u, in0=y2, scalar=0.0, in1=xdot,
                            op0=alu.is_equal, op1=alu.bypass)
                        nc.vector.scalar_tensor_tensor(
                            out=deg, in0=xdot, scalar=0.0, in1=u,
                            op0=alu.is_equal, op1=alu.mult)
                        nc.vector.tensor_tensor(out=xdot, in0=xdot, in1=deg, op=alu.add)
                        nc.vector.reciprocal(rec, xdot)
                        nc.vector.tensor_tensor(out=u, in0=ynorm, in1=rec, op=alu.mult)
                        nc.scalar.activation(at, u, AF.Arctan)
                        nc.vector.tensor_scalar(out=v, in0=xdot, scalar1=0.0,
                                                scalar2=None, op0=alu.is_lt)
                        nc.vector.scalar_tensor_tensor(
                            out=u, in0=v, scalar=PI, in1=at, op0=alu.mult, op1=alu.add)
                        nc.vector.tensor_scalar(out=out_ap, in0=u, scalar1=wc,
                                                scalar2=None, op0=alu.mult)

                    NI = [cp(3), cp(4), cp(5)]
                    NJ = [gp(3), gp(4), gp(5)]
                    emit_angle(fp(10), NI, rel)
                    emit_angle(fp(11), NJ, rel)
                    emit_angle(fp(12), NI, NJ)
                    nc.vector.memset(fp(13), 1.0)

                    # transpose planes -> (14, 32768) fp16 rows via PE
                    f14rows = bigA.tile([14, PAIRS], F16, tag="hbuf", name="f14rows")
                    fplTT = fpl[:].rearrange("p (r f) -> p f r", r=14)
                    id128 = cstm[:, M_ID128:M_ID128 + 128]
                    with tc.tile_pool(name="ptr2", bufs=4, space="PSUM") as ptr2:
                        for b64 in range(64):
                            pt = ptr2.tile([14, 512], f32, tag="pt", name="pt")
                            for ft in range(4):
                                f = 4 * b64 + ft
                                nc.tensor.transpose(
                                    pt[:, 128 * ft:128 * ft + 128],
                                    fplTT[:, f, :], id128)
                            if b64 % 2 == 0:
                                nc.scalar.copy(
                                    f14rows[:, 512 * b64:512 * (b64 + 1)], pt[:])
                            else:
                                nc.vector.tensor_copy(
                                    f14rows[:, 512 * b64:512 * (b64 + 1)], pt[:])

                    # M14 moment accumulation (PE)
                    pm = psmall.tile([14, 14], f32, tag="ps", name="pm")
                    fplT = fpl[:].rearrange("p (r f) -> p f r", r=14)
                    for col in range(256):
                        nc.tensor.matmul(pm[:], fplT[:, col, :], fplT[:, col, :],
                                         start=(col == 0), stop=(col == 255))
                    msb = statbuf[0:14, S_MSB:S_MSB + 14]
                    nc.scalar.copy(msb, pm[:])
                    mflat = statbuf[0:112, S_MFLAT:S_MFLAT + 2]
                    nc.sync.dma_start(mflat[:, 0:1], statbuf[0:8, S_MSB:S_MSB + 14])
                    nc.sync.dma_start(mflat[0:84, 1:2], statbuf[8:14, S_MSB:S_MSB + 14])
                    A0 = cst[0:112, C_A0T:C_A0T + 32]
                    psq0 = psmall.tile([8, 1], f32, tag="ps", name="psq0")
                    nc.tensor.matmul(psq0[:], A0[:, 0:8], mflat[:, 0:1],
                                     start=True, stop=False)
                    nc.tensor.matmul(psq0[:], A0[0:84, 16:24], mflat[0:84, 1:2],
                                     start=False, stop=True)
                    pss0 = psmall.tile([8, 1], f32, tag="ps", name="pss0")
                    nc.tensor.matmul(pss0[:], A0[:, 8:16], mflat[:, 0:1],
                                     start=True, stop=False)
                    nc.tensor.matmul(pss0[:], A0[0:84, 24:32], mflat[0:84, 1:2],
                                     start=False, stop=True)
                    sqq0 = statbuf[0:8, S_LAY + 10:S_LAY + 11]
                    nc.scalar.copy(sqq0, psq0[:])
                    sqs0 = statbuf[0:8, S_LAY + 11:S_LAY + 12]
                    nc.scalar.copy(sqs0, pss0[:])
                # ---- midB closed ----

                q80, s80 = allreduce_stats(sqq0, sqs0, 0)
                scale0, shift0 = gn_affine(q80, s80, CNT01, C_G0, C_B0, C_GE8, 8, 64, 0)

                with tc.tile_pool(name="bigB", bufs=1) as bigB:
                    # conv0 -> y0 (fp16)
                    y0sb = bigB.tile([64, PAIRS], F16, tag="ybuf", name="y0sb")
                    with tc.tile_pool(name="pconv", bufs=2, space="PSUM") as pconv:
                        for grp in range(32):
                            pc = pconv.tile([64, 1024], f32, tag="pc", name="pc")
                            for j in range(2):
                                c0 = 1024 * grp + 512 * j
                                nc.tensor.matmul(pc[:, 512 * j:512 * (j + 1)],
                                                 w0T, f14rows[:, c0:c0 + 512],
                                                 start=True, stop=True)
                            nc.scalar.copy(y0sb[:, 1024 * grp:1024 * (grp + 1)], pc[:])

                    h1 = bigA.tile([64, PAIRS], F16, tag="hbuf", name="h1")
                    sh1 = statbuf[0:64, S_SH1:S_SH1 + 1]
                    nc.scalar.activation(h1[:], y0sb[:], AF.Relu, bias=shift0,
                                         scale=scale0, accum_out=sh1)

                    # conv1 -> y1; stats1
                    y1sb = bigB.tile([64, PAIRS], F16, tag="ybuf", name="y1sb")
                    with tc.tile_pool(name="pconv1", bufs=2, space="PSUM") as pconv1:
                        for grp in range(32):
                            pc = pconv1.tile([64, 1024], f32, tag="pc1", name="pc1")
                            for j in range(2):
                                c0 = 1024 * grp + 512 * j
                                nc.tensor.matmul(pc[:, 512 * j:512 * (j + 1)],
                                                 w1T, h1[:, c0:c0 + 512],
                                                 start=True, stop=True)
                            nc.scalar.copy(y1sb[:, 1024 * grp:1024 * (grp + 1)], pc[:])
                    q1parts = statbuf[0:64, S_Q1P:S_Q1P + 16]
                    with tc.tile_pool(name="dmp", bufs=2) as dmp:
                        for grp in range(16):
                            dumpt = dmp.tile([64, 2048], F16, tag="dump", name="dump")
                            nc.scalar.activation(dumpt[:],
                                                 y1sb[:, 2048 * grp:2048 * (grp + 1)],
                                                 AF.Square,
                                                 accum_out=q1parts[:, grp:grp + 1])
                    q1ch = statbuf[0:64, S_Q1CH:S_Q1CH + 1]
                    nc.vector.tensor_reduce(q1ch, q1parts, axis=AX.X, op=alu.add)
                    psq1 = psmall.tile([8, 1], f32, tag="ps", name="psq1")
                    nc.tensor.matmul(psq1[:], cc(C_GI1, 8, rows=64), q1ch,
                                     start=True, stop=True)
                    pss1 = psmall.tile([8, 1], f32, tag="ps", name="pss1")
                    nc.tensor.matmul(pss1[:], cc(C_GW1, 8, rows=64), sh1,
                                     start=True, stop=True)
                    sqq1 = statbuf[0:8, S_LAY + 12 + 10:S_LAY + 12 + 11]
                    nc.scalar.copy(sqq1, psq1[:])
                    sqs1 = statbuf[0:8, S_LAY + 12 + 11:S_LAY + 12 + 12]
                    nc.scalar.copy(sqs1, pss1[:])
                    q81, s81 = allreduce_stats(sqq1, sqs1, 1)
                    scale1, shift1 = gn_affine(q81, s81, CNT01, C_G1, C_B1, C_GE8, 8, 64, 1)

                    h2 = bigA.tile([64, PAIRS], F16, tag="hbuf", name="h2")
                    sh2 = statbuf[0:64, S_SH2:S_SH2 + 1]
                    nc.scalar.activation(h2[:], y1sb[:], AF.Relu, bias=shift1,
                                         scale=scale1, accum_out=sh2)

                    # conv2; max over K; stats2
                    q2parts = statbuf[0:128, S_Q2P:S_Q2P + 32]
                    with tc.tile_pool(name="pconv2", bufs=2, space="PSUM") as pconv2, \
                         tc.tile_pool(name="dmp2", bufs=2) as dmp2:
                        for grp in range(32):
                            pc = pconv2.tile([128, 1024], f32, tag="pc2", name="pc2")
                            for j in range(2):
                                c0 = 1024 * grp + 512 * j
                                nc.tensor.matmul(pc[:, 512 * j:512 * (j + 1)],
                                                 w2T, h2[:, c0:c0 + 512],
                                                 start=True, stop=True)
                            rin = pc[:].rearrange("p (t s c) -> p t c s", t=2, s=32)
                            nc.vector.tensor_reduce(
                                outmax[:, 32 * grp:32 * (grp + 1)].rearrange(
                                    "p (t c) -> p t c", t=2),
                                rin, axis=AX.X, op=alu.max)
                            dumpt = dmp2.tile([128, 1024], F16, tag="dump2",
                                              name="dump2")
                            nc.scalar.activation(dumpt[:], pc[:], AF.Square,
                                                 accum_out=q2parts[:, grp:grp + 1])
                    q2ch = statbuf[0:128, S_Q2CH:S_Q2CH + 1]
                    nc.vector.tensor_reduce(q2ch, q2parts, axis=AX.X, op=alu.add)
                    psq2 = psmall.tile([8, 1], f32, tag="ps", name="psq2")
                    nc.tensor.matmul(psq2[:], cc(C_GI2, 8), q2ch, start=True, stop=True)
                    pss2 = psmall.tile([8, 1], f32, tag="ps", name="pss2")
                    nc.tensor.matmul(pss2[:], cc(C_GW2, 8, rows=64), sh2,
                                     start=True, stop=True)
                    sqq2 = statbuf[0:8, S_LAY + 24 + 10:S_LAY + 24 + 11]
                    nc.scalar.copy(sqq2, psq2[:])
                    sqs2 = statbuf[0:8, S_LAY + 24 + 11:S_LAY + 24 + 12]
                    nc.scalar.copy(sqs2, pss2[:])
                    q82, s82 = allreduce_stats(sqq2, sqs2, 2)
                    scale2, shift2 = gn_affine(q82, s82, CNT2, C_G2, C_B2, C_GE16, 8, 128, 2)

                    nc.scalar.activation(outx[:], outmax[:], AF.Identity,
                                         bias=shift2, scale=scale2)
                    nc.scalar.activation(outr[:], outx[:], AF.Relu)
                    nc.sync.dma_start(outx_d.ap(), outx[:])
                    nc.sync.dma_start(outr_d.ap(), outr[:])

    nc.compile()
    return nc


# ======================= host-side prep =======================

def _out_perm():
    col = np.arange(NCEN)
    q = col // 128
    rr = col % 128
    t = rr // 16
    c16 = col % 16
    return 128 * t + 16 * q + c16


def prep_core_inputs(core, inp):
    f32 = np.float32
    b = core // 4
    kq = core % 4
    xyz = np.asarray(inp['xyz'], f32)[b]
    feat = np.asarray(inp['feature'], f32)[b]
    cen = xyz[1024 * kq:1024 * (kq + 1)]
    cfeat = feat[1024 * kq:1024 * (kq + 1)]

    d = {}
    d['cen4T'] = np.concatenate([-2.0 * cen.T, np.ones((1, NCEN), f32)], 0).astype(f32)
    d['pts4'] = np.concatenate([xyz.T, (xyz * xyz).sum(-1)[None, :]], 0).astype(f32)
    src = np.zeros((P, N), f32)
    for q in range(8):
        for r in range(3):
            src[16 * q + r] = xyz[:, r]
            src[16 * q + 3 + r] = feat[:, r]
    d['src16'] = src
    pf = np.arange(P)[:, None]
    ff = np.arange(256)[None, :]
    qq = ff // 32
    ii = 128 * (ff % 32) + pf
    cenidx = 128 * (ii // 512) + 16 * qq + (ii % 16)
    cpl = np.zeros((P, 6 * 256), f32)
    for r in range(3):
        cpl[:, 256 * r:256 * (r + 1)] = cen[cenidx, r]
        cpl[:, 256 * (3 + r):256 * (4 + r)] = cfeat[cenidx, r]
    d['cpl'] = cpl

    w0 = np.asarray(inp['conv0_w'], f32)
    w1 = np.asarray(inp['conv1_w'], f32)
    w2 = np.asarray(inp['conv2_w'], f32)

    # cst
    cst = np.zeros((P, CST_W), f32)
    cst[:, C_NEG1] = -1.0
    for g in range(8):
        cst[0:64, C_GW1 + g] = w1[8 * g:8 * g + 8].sum(0)
        cst[0:64, C_GW2 + g] = w2[16 * g:16 * g + 16].sum(0)
    cst[0:64, C_GI1:C_GI1 + 8] = (np.arange(64)[:, None] // 8 == np.arange(8)[None, :])
    cst[0:128, C_GI2:C_GI2 + 8] = (np.arange(128)[:, None] // 16 == np.arange(8)[None, :])
    cst[0:8, C_GE8:C_GE8 + 64] = (np.arange(64)[None, :] // 8 == np.arange(8)[:, None])
    cst[0:8, C_GE16:C_GE16 + 128] = (np.arange(128)[None, :] // 16 == np.arange(8)[:, None])
    A = np.zeros((196, 16), f32)
    for g in range(8):
        Qg = np.zeros((14, 14), f32)
        ug = np.zeros(14, f32)
        for c in range(8 * g, 8 * g + 8):
            Qg[:13, :13] += np.outer(w0[c], w0[c])
            ug[:13] += w0[c]
        A[:, g] = Qg.reshape(-1)
        Ug = np.zeros((14, 14), f32)
        Ug[:, 13] = ug
        A[:, 8 + g] = Ug.reshape(-1)
    cst[0:112, C_A0T:C_A0T + 16] = A[0:112]
    cst[0:84, C_A0T + 16:C_A0T + 32] = A[112:196]
    cst[0:64, C_G0] = np.asarray(inp['gn0_g'], f32).reshape(-1)
    cst[0:64, C_B0] = np.asarray(inp['gn0_b'], f32).reshape(-1)
    cst[0:64, C_G1] = np.asarray(inp['gn1_g'], f32).reshape(-1)
    cst[0:64, C_B1] = np.asarray(inp['gn1_b'], f32).reshape(-1)
    cst[0:128, C_G2] = np.asarray(inp['gn2_g'], f32).reshape(-1)
    cst[0:128, C_B2] = np.asarray(inp['gn2_b'], f32).reshape(-1)
    cst[0:2, C_BSELC] = np.array([1.0, 0.0], f32) if b == 0 else np.array([0.0, 1.0], f32)
    cst[0:16, C_BS0] = 1.0 if b == 0 else 0.0
    cst[0:16, C_BS1] = 1.0 if b == 1 else 0.0
    d['cst'] = cst

    # cstm
    cstm = np.zeros((P, CSTM_W), f32)
    cn = (cen * cen).sum(-1)
    cstm[:, M_THR:M_THR + 8] = R2 - cn.reshape(TILES, 128).T
    cstm[:, M_IOTA32:M_IOTA32 + 32] = np.arange(32, dtype=f32)[None, :]
    cstm[:, M_ID:M_ID + 64] = np.tile(np.eye(64, dtype=f32), (2, 1))
    cstm[:, M_WA] = np.asarray(inp['wa'], f32).reshape(-1)[0]
    cstm[:, M_WB] = np.asarray(inp['wb'], f32).reshape(-1)[0]
    cstm[:, M_WC] = np.asarray(inp['wc'], f32).reshape(-1)[0]
    cstm[:, M_ID128:M_ID128 + 128] = np.eye(128, dtype=f32)
    d['cstm'] = cstm

    # cstw (fp16)
    cstw = np.zeros((P, 256), np.float16)
    cstw[0:13, 0:64] = w0.T.astype(np.float16)
    cstw[0:64, 64:128] = w1.T.astype(np.float16)
    cstw[0:64, 128:256] = w2.T.astype(np.float16)
    d['cstw'] = cstw
    return d


_NC_CACHE = {}


def kernel(**inputs):
    if 'nc' not in _NC_CACHE:
        _NC_CACHE['nc'] = build_program()
    nc = _NC_CACHE['nc']
    in_maps = [prep_core_inputs(c, inputs) for c in range(8)]
    res = bass_utils.run_bass_kernel_spmd(nc, in_maps, core_ids=list(range(8)))
    perm = _out_perm()
    out_r = np.zeros((B, 128, N), np.float32)
    out_x = np.zeros((B, 128, N), np.float32)
    for c in range(8):
        b = c // 4
        kq = c % 4
        out_x[b][:, 1024 * kq + perm] = res.results[c]['outx']
        out_r[b][:, 1024 * kq + perm] = res.results[c]['outr']
    return (out_r, out_x)


# revision 29
# speedup vs baseline: 2197.5161x; 2197.5161x over previous
"""Trainium2 Bass kernel for nn_LocalFeatue (PPF local feature module).

Shards the N (center) axis x batch across 8 NeuronCores: core c handles
batch c//4, centers [(c%4)*1024, (c%4+1)*1024).  Ball query, gather, PPF
features, 3x (1x1 conv + GroupNorm) and K-maxpool all run on device; the
GroupNorm statistics are allreduced across cores (tiny 2x16 f32 tensors).

Per-core pair list (per GPSIMD core q, one tile of 128 centers each):
list position i = 512*t + 16*s + c16 maps to (tile t, slot s, center
16*q + c16 of tile t).  Pair-plane mapping: partition p = i % 128,
plane column f = 32*q + i // 128.  Conv column = 4096*q + i.
Output column = 128*q + 16*t + c16 -> center 128*t + 16*q + c16
(host permutes).

SBUF tiles pad to 4KB/partition, so small constants and scratch are
packed into a few column-sliced tiles (cst/cstw/cstm/statbuf/scales).
"""

import sys
sys.path.insert(0, '/opt/trn_rl_repo')

import numpy as np
import concourse.bacc as bacc
import concourse.bass as bass
import concourse.mybir as mybir
import concourse.tile as tile
import concourse.bass_utils as bass_utils

dt = mybir.dt
alu = mybir.AluOpType
AF = mybir.ActivationFunctionType
AX = mybir.AxisListType

B, N, K = 2, 4096, 32
R2 = np.float32(0.1 * 0.1)
NCEN = 1024
TILES = 8
P = 128
PAIRS = NCEN * K
EPS = 1e-5
CNT01 = float(8 * K * N)
CNT2 = float(16 * K * N)
PI = float(np.pi)
F16 = dt.float16

# cst (f32, 128 x 320) column map
C_NEG1 = 0
C_GW1 = 1       # (64, 8)
C_GW2 = 9       # (64, 8)
C_GI1 = 17      # (64, 8)
C_GI2 = 25      # (128, 8)
C_GE8 = 33      # (8, 64)
C_GE16 = 97     # (8, 128)
C_A0T = 225     # (112, 32)
C_G0, C_B0, C_G1, C_B1, C_G2, C_B2 = 257, 258, 259, 260, 261, 262
C_BSELC = 263   # (2, 1)
C_BS0 = 264     # (16, 1)
C_BS1 = 265     # (16, 1)
CST_W = 320

# cstm (f32, 128 x 128) column map (mid-phase consts)
M_THR = 0       # (128, 8)
M_IOTA32 = 8    # (128, 32)
M_ID = 40       # (128, 64) 2-stacked 64x64 identity
M_WA, M_WB, M_WC = 104, 105, 106
M_ID128 = 128   # (128, 128) identity
CSTM_W = 256

# statbuf (f32, 128 x 128) column map (kernel-written smalls)
S_SH1 = 0
S_SH2 = 1
S_Q1CH = 2
S_Q2CH = 3
S_Q1P = 4       # (64, 16)
S_Q2P = 20      # (128, 32)
S_MSB = 52      # (14, 14)
S_LAY = 66      # per layer L: 12 cols at 66+12L: mean8,ex2,m2,var8,sd8,rsd8,
                #   t0q,t1q,t0s,t1s,sqq,sqs
S_MFLAT = 102   # (112, 2)
S_AR2 = 104     # (2, 16)
S_SROW = 120    # per layer L: q at 120+2L, s at 121+2L
S_TMP = 126     # 126+layer (64/128,1) tmp
STAT_W = 136

# scales (f32, 128 x 16): per layer L: rsdC 4L, meanC 4L+1, scale 4L+2, shift 4L+3


def build_program(skip_collective=False):
    nc = bacc.Bacc("TRN2", target_bir_lowering=False, debug=False,
                   enable_asserts=False, num_devices=8)
    f32 = dt.float32

    def din(name, shape, dtype=f32):
        return nc.dram_tensor(name, shape, dtype, kind="ExternalInput")

    cen4T_d = din("cen4T", [4, NCEN])
    pts4_d = din("pts4", [4, N])
    src16_d = din("src16", [P, N])
    cpl_d = din("cpl", [P, 6 * 256])
    cst_d = din("cst", [P, CST_W])
    cstm_d = din("cstm", [P, CSTM_W])
    cstw_d = din("cstw", [P, 256], F16)

    outx_d = nc.dram_tensor("outx", [128, NCEN], f32, kind="ExternalOutput")
    outr_d = nc.dram_tensor("outr", [128, NCEN], f32, kind="ExternalOutput")

    with tile.TileContext(nc) as tc:
        with tc.tile_pool(name="per", bufs=1) as per, \
             tc.tile_pool(name="midout", bufs=1) as midout, \
             tc.tile_pool(name="dram", bufs=1, space="DRAM") as dram, \
             tc.tile_pool(name="psmall", bufs=2, space="PSUM") as psmall:

            cst = per.tile([P, CST_W], f32, tag="cst")
            nc.sync.dma_start(cst[:], cst_d.ap())
            cstw = per.tile([P, 256], F16, tag="cstw")
            nc.sync.dma_start(cstw[:], cstw_d.ap())
            statbuf = per.tile([P, STAT_W], f32, tag="statbuf")
            scales = per.tile([P, 16], f32, tag="scales")
            outmax = per.tile([P, NCEN], f32, tag="outmax")
            outx = per.tile([P, NCEN], f32, tag="outx")
            outr = per.tile([P, NCEN], f32, tag="outr")

            def cc(col, w=1, rows=128, r0=0):
                return cst[r0:r0 + rows, col:col + w]

            w0T = cstw[0:14, 0:64]
            w1T = cstw[0:64, 64:128]
            w2T = cstw[0:64, 128:256]

            idxw = midout.tile([P, 256], dt.int16, tag="idxw")
            cstm = midout.tile([P, CSTM_W], f32, tag="cstm")
            nc.sync.dma_start(cstm[:], cstm_d.ap())

            # ---------- helpers ----------
            def allreduce_stats(qt, st, layer):
                """qt, st: (8,1) APs of partial sums -> [q8, s8] (8,1) APs."""
                L0 = S_LAY + 12 * layer
                t0q = statbuf[0:8, L0 + 6:L0 + 7]
                t1q = statbuf[0:8, L0 + 7:L0 + 8]
                t0s = statbuf[0:8, L0 + 8:L0 + 9]
                t1s = statbuf[0:8, L0 + 9:L0 + 10]
                bs0 = cc(C_BS0, rows=8)
                bs1 = cc(C_BS1, rows=8)
                nc.vector.tensor_scalar(out=t0q, in0=qt, scalar1=bs0, scalar2=None, op0=alu.mult)
                nc.vector.tensor_scalar(out=t1q, in0=qt, scalar1=bs1, scalar2=None, op0=alu.mult)
                nc.vector.tensor_scalar(out=t0s, in0=st, scalar1=bs0, scalar2=None, op0=alu.mult)
                nc.vector.tensor_scalar(out=t1s, in0=st, scalar1=bs1, scalar2=None, op0=alu.mult)
                arin = dram.tile([2, 16], f32, tag=f"arin{layer}", name=f"arin{layer}")
                arout = dram.tile([2, 16], f32, tag=f"arout{layer}", name=f"arout{layer}")
                nc.sync.dma_start(arin[0:1, 0:8], t0q)
                nc.sync.dma_start(arin[1:2, 0:8], t1q)
                nc.sync.dma_start(arin[0:1, 8:16], t0s)
                nc.sync.dma_start(arin[1:2, 8:16], t1s)
                if skip_collective:
                    nc.sync.dma_start(arout[:], arin[:])
                else:
                    nc.gpsimd.collective_compute(
                        "AllReduce", alu.add, replica_groups=[list(range(8))],
                        ins=[arin[:]], outs=[arout[:]])
                ar2 = statbuf[0:2, S_AR2:S_AR2 + 16]
                nc.sync.dma_start(ar2, arout[:])
                outs = []
                for half in (0, 1):
                    psr = psmall.tile([8, 1], f32, tag="ps", name=f"psr{layer}_{half}")
                    nc.tensor.matmul(psr[:], ar2[:, 8 * half:8 * half + 8],
                                     cc(C_BSELC, rows=2), start=True, stop=True)
                    srow = statbuf[0:8, S_SROW + 2 * layer + half:S_SROW + 2 * layer + half + 1]
                    nc.scalar.copy(srow, psr[:])
                    outs.append(srow)
                return outs

            def gn_affine(q8, s8, cnt, gcol, bcol, gecol, gerows, C, layer):
                L0 = S_LAY + 12 * layer
                mean8 = statbuf[0:8, L0 + 0:L0 + 1]
                ex2 = statbuf[0:8, L0 + 1:L0 + 2]
                m2 = statbuf[0:8, L0 + 2:L0 + 3]
                var8 = statbuf[0:8, L0 + 3:L0 + 4]
                sd8 = statbuf[0:8, L0 + 4:L0 + 5]
                rsd8 = statbuf[0:8, L0 + 5:L0 + 6]
                nc.vector.tensor_scalar(out=mean8, in0=s8, scalar1=1.0 / cnt,
                                        scalar2=None, op0=alu.mult)
                nc.vector.tensor_scalar(out=ex2, in0=q8, scalar1=1.0 / cnt,
                                        scalar2=None, op0=alu.mult)
                nc.vector.tensor_tensor(out=m2, in0=mean8, in1=mean8, op=alu.mult)
                nc.vector.tensor_tensor(out=var8, in0=ex2, in1=m2, op=alu.subtract)
                nc.vector.tensor_scalar(out=var8, in0=var8, scalar1=EPS,
                                        scalar2=None, op0=alu.add)
                nc.scalar.activation(sd8, var8, AF.Sqrt)
                nc.vector.reciprocal(rsd8, sd8)
                gexp = cst[0:8, gecol:gecol + C]
                rsdC = scales[0:C, 4 * layer:4 * layer + 1]
                meanC = scales[0:C, 4 * layer + 1:4 * layer + 2]
                pex = psmall.tile([C, 1], f32, tag="ps2", name=f"pex{layer}")
                nc.tensor.matmul(pex[:], gexp, rsd8, start=True, stop=True)
                nc.scalar.copy(rsdC, pex[:])
                pex2 = psmall.tile([C, 1], f32, tag="ps2", name=f"pex2{layer}")
                nc.tensor.matmul(pex2[:], gexp, mean8, start=True, stop=True)
                nc.scalar.copy(meanC, pex2[:])
                scale = scales[0:C, 4 * layer + 2:4 * layer + 3]
                shift = scales[0:C, 4 * layer + 3:4 * layer + 4]
                tmp = statbuf[0:C, S_TMP + layer:S_TMP + layer + 1]
                nc.vector.tensor_tensor(out=scale, in0=cc(gcol, rows=C), in1=rsdC, op=alu.mult)
                nc.vector.tensor_tensor(out=tmp, in0=meanC, in1=scale, op=alu.mult)
                nc.vector.tensor_tensor(out=shift, in0=cc(bcol, rows=C), in1=tmp, op=alu.subtract)
                return scale, shift

            with tc.tile_pool(name="bigA", bufs=1) as bigA:
                # ================= ball-query era =================
                with tc.tile_pool(name="midA", bufs=1) as midA:
                    iota16 = midA.tile([P, N], dt.int16, tag="iota16")
                    nc.gpsimd.iota(iota16[:], pattern=[[1, N]], base=0,
                                   channel_multiplier=0)
                    idx48 = midA.tile([P, 48 * TILES], dt.int16, tag="idx48")
                    cnts = midA.tile([P, TILES], f32, tag="cnts")
                    cen4T = midA.tile([4, NCEN], f32, tag="cen4T")
                    nc.sync.dma_start(cen4T[:], cen4T_d.ap())
                    pts4 = midA.tile([4, N], f32, tag="pts4")
                    nc.sync.dma_start(pts4[:], pts4_d.ap())

                    with tc.tile_pool(name="ball", bufs=2) as ball, \
                         tc.tile_pool(name="pdist", bufs=2, space="PSUM") as pdist:
                        for t in range(TILES):
                            mask = ball.tile([P, N], dt.bfloat16, tag="mask")
                            for h in range(4):
                                pd = pdist.tile([P, 1024], f32, tag="pd", name="pd")
                                for j in range(2):
                                    c0 = 1024 * h + 512 * j
                                    nc.tensor.matmul(
                                        pd[:, 512 * j:512 * (j + 1)],
                                        cen4T[:, 128 * t:128 * (t + 1)],
                                        pts4[:, c0:c0 + 512],
                                        start=True, stop=True)
                                nc.vector.tensor_scalar(
                                    out=mask[:, 1024 * h:1024 * (h + 1)],
                                    in0=pd[:], scalar1=cstm[:, M_THR + t:M_THR + t + 1],
                                    scalar2=None, op0=alu.is_le)
                            rank = ball.tile([P, N], f32, tag="rank")
                            nc.vector.tensor_tensor_scan(
                                out=rank[:], data0=mask[:], data1=mask[:],
                                initial=0.0, op0=alu.add, op1=alu.max)
                            nc.scalar.copy(cnts[:, t:t + 1], rank[:, N - 1:N])
                            sel = ball.tile([P, N], dt.bfloat16, tag="sel")
                            nc.vector.scalar_tensor_tensor(
                                out=sel[:], in0=rank[:], scalar=47.0, in1=mask[:],
                                op0=alu.min, op1=alu.mult)
                            slot16 = ball.tile([P, N], dt.int16, tag="slot16")
                            nc.scalar.activation(slot16[:], sel[:], AF.Identity,
                                                 bias=cc(C_NEG1))
                            nc.gpsimd.local_scatter(
                                idx48[:, 48 * t:48 * t + 48], iota16[:], slot16[:],
                                channels=128, num_elems=48, num_idxs=N)
                            # pad-fix
                            bsc = ball.tile([P, 160], f32, tag="bsc")
                            pmf = bsc[:, 0:32]
                            firstf = bsc[:, 32:33]
                            idxf = bsc[:, 48:80]
                            dtl = bsc[:, 80:112]
                            etl = bsc[:, 112:144]
                            nc.vector.tensor_scalar(
                                out=pmf, in0=cstm[:, M_IOTA32:M_IOTA32 + 32],
                                scalar1=cnts[:, t:t + 1], scalar2=None, op0=alu.is_lt)
                            nc.scalar.copy(firstf, idx48[:, 48 * t:48 * t + 1])
                            nc.scalar.copy(idxf, idx48[:, 48 * t:48 * t + 32])
                            nc.vector.tensor_scalar(
                                out=dtl, in0=idxf, scalar1=firstf,
                                scalar2=None, op0=alu.subtract)
                            nc.vector.tensor_tensor(out=etl, in0=dtl, in1=pmf,
                                                    op=alu.mult)
                            nc.vector.tensor_scalar(
                                out=idxw[:, 32 * t:32 * t + 32], in0=etl,
                                scalar1=firstf, scalar2=None, op0=alu.add)

                # ================= gather/feature era =================
                with tc.tile_pool(name="midB", bufs=1) as midB:
                    src16 = midB.tile([P, N], f32, tag="m2", name="src16")
                    nc.sync.dma_start(src16[:], src16_d.ap())
                    gath = midB.tile([P, N], f32, tag="mbig", name="gath")
                    nc.gpsimd.ap_gather(
                        gath[:].rearrange("p (n d) -> p n d", d=1),
                        src16[:].rearrange("p (n d) -> p n d", d=1),
                        idxw[:], channels=128, num_elems=N, d=1, num_idxs=4096)

                    gpl = midB.tile([P, 6 * 256], f32, tag="gpl")
                    with tc.tile_pool(name="ptr", bufs=2, space="PSUM") as ptr:
                        for Q in range(2):
                            for half in range(2):
                                ptile = ptr.tile([P, 1024], f32, tag="ptile",
                                                 name="ptile")
                                for jj in range(16):
                                    j = 16 * half + jj
                                    nc.tensor.transpose(
                                        ptile[:, 64 * jj:64 * jj + 64],
                                        gath[64 * Q:64 * Q + 64,
                                             128 * j:128 * j + 128],
                                        cstm[64 * Q:64 * Q + 64, M_ID:M_ID + 64])
                                for sub in range(4):
                                    g = 4 * Q + sub
                                    src_ap = ptile[:].rearrange(
                                        "p (j r) -> p j r", j=16)[:, :, 16 * sub:16 * sub + 6]
                                    dst_ap = gpl[:].rearrange(
                                        "p (r g j) -> p g j r", r=6, g=8)[
                                        :, g, 16 * half:16 * half + 16, :]
                                    nc.vector.tensor_copy(dst_ap, src_ap)

                    cpl = midB.tile([P, 6 * 256], f32, tag="cpl")
                    nc.sync.dma_start(cpl[:], cpl_d.ap())
                    fsc = midB.tile([P, 17 * 256], f32, tag="fsc")
                    fpl = midB.tile([P, 14 * 256], f32, tag="fpl")

                    def gp(r):
                        return gpl[:, 256 * r:256 * (r + 1)]

                    def cp(r):
                        return cpl[:, 256 * r:256 * (r + 1)]

                    def fp(r):
                        return fpl[:, 256 * r:256 * (r + 1)]

                    def sc(r):
                        return fsc[:, 256 * r:256 * (r + 1)]

                    wa = cstm[:, M_WA:M_WA + 1]
                    wb = cstm[:, M_WB:M_WB + 1]
                    wc = cstm[:, M_WC:M_WC + 1]
                    for r in range(3):
                        nc.vector.tensor_scalar(out=fp(r), in0=cp(r),
                                                scalar1=wa, scalar2=None, op0=alu.mult)
                        nc.vector.tensor_scalar(out=fp(3 + r), in0=gp(r),
                                                scalar1=wa, scalar2=None, op0=alu.mult)
                    rel = [sc(0), sc(1), sc(2)]
                    for r in range(3):
                        nc.vector.tensor_tensor(out=rel[r], in0=gp(r), in1=cp(r),
                                                op=alu.subtract)
                        nc.vector.tensor_scalar(out=fp(6 + r), in0=rel[r],
                                                scalar1=wb, scalar2=None, op0=alu.mult)
                    d2, t2, dist = sc(3), sc(4), sc(5)
                    nc.vector.tensor_tensor(out=d2, in0=rel[0], in1=rel[0], op=alu.mult)
                    nc.vector.tensor_tensor(out=t2, in0=rel[1], in1=rel[1], op=alu.mult)
                    nc.vector.tensor_tensor(out=d2, in0=d2, in1=t2, op=alu.add)
                    nc.vector.tensor_tensor(out=t2, in0=rel[2], in1=rel[2], op=alu.mult)
                    nc.vector.tensor_tensor(out=d2, in0=d2, in1=t2, op=alu.add)
                    nc.scalar.activation(dist, d2, AF.Sqrt)
                    nc.vector.tensor_scalar(out=fp(9), in0=dist, scalar1=wb,
                                            scalar2=None, op0=alu.mult)

                    cx, cy, cz = sc(6), sc(7), sc(8)
                    u, v = sc(9), sc(10)
                    y2, ynorm, xdot = sc(11), sc(12), sc(13)
                    deg, rec, at = sc(14), sc(15), sc(16)

                    def emit_angle(out_ap, a, b):
                        nc.vector.tensor_tensor(out=u, in0=a[1], in1=b[2], op=alu.mult)
                        nc.vector.tensor_tensor(out=v, in0=a[2], in1=b[1], op=alu.mult)
                        nc.vector.tensor_tensor(out=cx, in0=u, in1=v, op=alu.subtract)
                        nc.vector.tensor_tensor(out=u, in0=a[2], in1=b[0], op=alu.mult)
                        nc.vector.tensor_tensor(out=v, in0=a[0], in1=b[2], op=alu.mult)
                        nc.vector.tensor_tensor(out=cy, in0=u, in1=v, op=alu.subtract)
                        nc.vector.tensor_tensor(out=u, in0=a[0], in1=b[1], op=alu.mult)
                        nc.vector.tensor_tensor(out=v, in0=a[1], in1=b[0], op=alu.mult)
                        nc.vector.tensor_tensor(out=cz, in0=u, in1=v, op=alu.subtract)
                        nc.vector.tensor_tensor(out=y2, in0=cx, in1=cx, op=alu.mult)
                        nc.vector.tensor_tensor(out=u, in0=cy, in1=cy, op=alu.mult)
                        nc.vector.tensor_tensor(out=y2, in0=y2, in1=u, op=alu.add)
                        nc.vector.tensor_tensor(out=u, in0=cz, in1=cz, op=alu.mult)
                        nc.vector.tensor_tensor(out=y2, in0=y2, in1=u, op=alu.add)
                        nc.scalar.activation(ynorm, y2, AF.Sqrt)
                        nc.vector.tensor_tensor(out=xdot, in0=a[0], in1=b[0], op=alu.mult)
                        nc.vector.tensor_tensor(out=u, in0=a[1], in1=b[1], op=alu.mult)
                        nc.vector.tensor_tensor(out=xdot, in0=xdot, in1=u, op=alu.add)
                        nc.vector.tensor_tensor(out=u, in0=a[2], in1=b[2], op=alu.mult)
                        nc.vector.tensor_tensor(out=xdot, in0=xdot, in1=u, op=alu.add)
                        nc.vector.scalar_tensor_tensor(
                            out=u, in0=y2, scalar=0.0, in1=xdot,
                            op0=alu.is_equal, op1=alu.bypass)
                        nc.vector.scalar_tensor_tensor(
                            out=deg, in0=xdot, scalar=0.0, in1=u,
                            op0=alu.is_equal, op1=alu.mult)
                        nc.vector.tensor_tensor(out=xdot, in0=xdot, in1=deg, op=alu.add)
                        nc.vector.reciprocal(rec, xdot)
                        nc.vector.tensor_tensor(out=u, in0=ynorm, in1=rec, op=alu.mult)
                        nc.scalar.activation(at, u, AF.Arctan)
                        nc.vector.tensor_scalar(out=v, in0=xdot, scalar1=0.0,
                                                scalar2=None, op0=alu.is_lt)
                        nc.vector.scalar_tensor_tensor(
                            out=u, in0=v, scalar=PI, in1=at, op0=alu.mult, op1=alu.add)
                        nc.vector.tensor_scalar(out=out_ap, in0=u, scalar1=wc,
                                                scalar2=None, op0=alu.mult)

                    NI = [cp(3), cp(4), cp(5)]
                    NJ = [gp(3), gp(4), gp(5)]
                    emit_angle(fp(10), NI, rel)
                    emit_angle(fp(11), NJ, rel)
                    emit_angle(fp(12), NI, NJ)
                    nc.vector.memset(fp(13), 1.0)

                    # transpose planes -> (14, 32768) fp16 rows via PE
                    f14rows = bigA.tile([14, PAIRS], F16, tag="hbuf", name="f14rows")
                    fplTT = fpl[:].rearrange("p (r f) -> p f r", r=14)
                    id128 = cstm[:, M_ID128:M_ID128 + 128]
                    with tc.tile_pool(name="ptr2", bufs=4, space="PSUM") as ptr2:
                        for b64 in range(64):
                            pt = ptr2.tile([14, 512], f32, tag="pt", name="pt")
                            for ft in range(4):
                                f = 4 * b64 + ft
                                nc.tensor.transpose(
                                    pt[:, 128 * ft:128 * ft + 128],
                                    fplTT[:, f, :], id128)
                            if b64 % 2 == 0:
                                nc.scalar.copy(
                                    f14rows[:, 512 * b64:512 * (b64 + 1)], pt[:])
                            else:
                                nc.vector.tensor_copy(
                                    f14rows[:, 512 * b64:512 * (b64 + 1)], pt[:])

                    # M14 moment accumulation (PE)
                    pm = psmall.tile([14, 14], f32, tag="ps", name="pm")
                    fplT = fpl[:].rearrange("p (r f) -> p f r", r=14)
                    for col in range(256):
                        nc.tensor.matmul(pm[:], fplT[:, col, :], fplT[:, col, :],
                                         start=(col == 0), stop=(col == 255))
                    msb = statbuf[0:14, S_MSB:S_MSB + 14]
                    nc.scalar.copy(msb, pm[:])
                    mflat = statbuf[0:112, S_MFLAT:S_MFLAT + 2]
                    nc.sync.dma_start(mflat[:, 0:1], statbuf[0:8, S_MSB:S_MSB + 14])
                    nc.sync.dma_start(mflat[0:84, 1:2], statbuf[8:14, S_MSB:S_MSB + 14])
                    A0 = cst[0:112, C_A0T:C_A0T + 32]
                    psq0 = psmall.tile([8, 1], f32, tag="ps", name="psq0")
                    nc.tensor.matmul(psq0[:], A0[:, 0:8], mflat[:, 0:1],
                                     start=True, stop=False)
                    nc.tensor.matmul(psq0[:], A0[0:84, 16:24], mflat[0:84, 1:2],
                                     start=False, stop=True)
                    pss0 = psmall.tile([8, 1], f32, tag="ps", name="pss0")
                    nc.tensor.matmul(pss0[:], A0[:, 8:16], mflat[:, 0:1],
                                     start=True, stop=False)
                    nc.tensor.matmul(pss0[:], A0[0:84, 24:32], mflat[0:84, 1:2],
                                     start=False, stop=True)
                    sqq0 = statbuf[0:8, S_LAY + 10:S_LAY + 11]
                    nc.scalar.copy(sqq0, psq0[:])
                    sqs0 = statbuf[0:8, S_LAY + 11:S_LAY + 12]
                    nc.scalar.copy(sqs0, pss0[:])
                # ---- midB closed ----

                q80, s80 = allreduce_stats(sqq0, sqs0, 0)
                scale0, shift0 = gn_affine(q80, s80, CNT01, C_G0, C_B0, C_GE8, 8, 64, 0)

                with tc.tile_pool(name="bigB", bufs=1) as bigB:
                    # conv0 -> y0 (fp16)
                    y0sb = bigB.tile([64, PAIRS], F16, tag="ybuf", name="y0sb")
                    with tc.tile_pool(name="pconv", bufs=2, space="PSUM") as pconv:
                        for grp in range(32):
                            pc = pconv.tile([64, 1024], f32, tag="pc", name="pc")
                            for j in range(2):
                                c0 = 1024 * grp + 512 * j
                                nc.tensor.matmul(pc[:, 512 * j:512 * (j + 1)],
                                                 w0T, f14rows[:, c0:c0 + 512],
                                                 start=True, stop=True)
                            nc.scalar.copy(y0sb[:, 1024 * grp:1024 * (grp + 1)], pc[:])

                    h1 = bigA.tile([64, PAIRS], F16, tag="hbuf", name="h1")
                    sh1 = statbuf[0:64, S_SH1:S_SH1 + 1]
                    nc.scalar.activation(h1[:], y0sb[:], AF.Relu, bias=shift0,
                                         scale=scale0, accum_out=sh1)

                    # conv1 -> y1; stats1
                    y1sb = bigB.tile([64, PAIRS], F16, tag="ybuf", name="y1sb")
                    with tc.tile_pool(name="pconv1", bufs=2, space="PSUM") as pconv1:
                        for grp in range(32):
                            pc = pconv1.tile([64, 1024], f32, tag="pc1", name="pc1")
                            for j in range(2):
                                c0 = 1024 * grp + 512 * j
                                nc.tensor.matmul(pc[:, 512 * j:512 * (j + 1)],
                                                 w1T, h1[:, c0:c0 + 512],
                                                 start=True, stop=True)
                            nc.scalar.copy(y1sb[:, 1024 * grp:1024 * (grp + 1)], pc[:])
                    q1parts = statbuf[0:64, S_Q1P:S_Q1P + 16]
                    with tc.tile_pool(name="dmp", bufs=2) as dmp:
                        for grp in range(16):
                            dumpt = dmp.tile([64, 2048], F16, tag="dump", name="dump")
                            nc.scalar.activation(dumpt[:],
                                                 y1sb[:, 2048 * grp:2048 * (grp + 1)],
                                                 AF.Square,
                                                 accum_out=q1parts[:, grp:grp + 1])
                    q1ch = statbuf[0:64, S_Q1CH:S_Q1CH + 1]
                    nc.vector.tensor_reduce(q1ch, q1parts, axis=AX.X, op=alu.add)
                    psq1 = psmall.tile([8, 1], f32, tag="ps", name="psq1")
                    nc.tensor.matmul(psq1[:], cc(C_GI1, 8, rows=64), q1ch,
                                     start=True, stop=True)
                    pss1 = psmall.tile([8, 1], f32, tag="ps", name="pss1")
                    nc.tensor.matmul(pss1[:], cc(C_GW1, 8, rows=64), sh1,
                                     start=True, stop=True)
                    sqq1 = statbuf[0:8, S_LAY + 12 + 10:S_LAY + 12 + 11]
                    nc.scalar.copy(sqq1, psq1[:])
                    sqs1 = statbuf[0:8, S_LAY + 12 + 11:S_LAY + 12 + 12]
                    nc.scalar.copy(sqs1, pss1[:])
                    q81, s81 = allreduce_stats(sqq1, sqs1, 1)
                    scale1, shift1 = gn_affine(q81, s81, CNT01, C_G1, C_B1, C_GE8, 8, 64, 1)

                    h2 = bigA.tile([64, PAIRS], F16, tag="hbuf", name="h2")
                    sh2 = statbuf[0:64, S_SH2:S_SH2 + 1]
                    nc.scalar.activation(h2[:], y1sb[:], AF.Relu, bias=shift1,
                                         scale=scale1, accum_out=sh2)

                    # conv2; max over K; stats2
                    q2parts = statbuf[0:128, S_Q2P:S_Q2P + 32]
                    with tc.tile_pool(name="pconv2", bufs=2, space="PSUM") as pconv2, \
                         tc.tile_pool(name="dmp2", bufs=2) as dmp2:
                        for grp in range(32):
                            pc = pconv2.tile([128, 1024], f32, tag="pc2", name="pc2")
                            for j in range(2):
                                c0 = 1024 * grp + 512 * j
                                nc.tensor.matmul(pc[:, 512 * j:512 * (j + 1)],
                                                 w2T, h2[:, c0:c0 + 512],
                                                 start=True, stop=True)
                            rin = pc[:].rearrange("p (t s c) -> p t c s", t=2, s=32)
                            nc.vector.tensor_reduce(
                                outmax[:, 32 * grp:32 * (grp + 1)].rearrange(
                                    "p (t c) -> p t c", t=2),
                                rin, axis=AX.X, op=alu.max)
                            dumpt = dmp2.tile([128, 1024], F16, tag="dump2",
                                              name="dump2")
                            nc.scalar.activation(dumpt[:], pc[:], AF.Square,
                                                 accum_out=q2parts[:, grp:grp + 1])
                    q2ch = statbuf[0:128, S_Q2CH:S_Q2CH + 1]
                    nc.vector.tensor_reduce(q2ch, q2parts, axis=AX.X, op=alu.add)
                    psq2 = psmall.tile([8, 1], f32, tag="ps", name="psq2")
                    nc.tensor.matmul(psq2[:], cc(C_GI2, 8), q2ch, start=True, stop=True)
                    pss2 = psmall.tile([8, 1], f32, tag="ps", name="pss2")
                    nc.tensor.matmul(pss2[:], cc(C_GW2, 8, rows=64), sh2,
                                     start=True, stop=True)
                    sqq2 = statbuf[0:8, S_LAY + 24 + 10:S_LAY + 24 + 11]
                    nc.scalar.copy(sqq2, psq2[:])
                    sqs2 = statbuf[0:8, S_LAY + 24 + 11:S_LAY + 24 + 12]
                    nc.scalar.copy(sqs2, pss2[:])
                    q82, s82 = allreduce_stats(sqq2, sqs2, 2)
                    scale2, shift2 = gn_affine(q82, s82, CNT2, C_G2, C_B2, C_GE16, 8, 128, 2)

                    nc.scalar.activation(outx[:], outmax[:], AF.Identity,
                                         bias=shift2, scale=scale2)
                    nc.scalar.activation(outr[:], outx[:], AF.Relu)
                    nc.sync.dma_start(outx_d.ap(), outx[:])
                    nc.sync.dma_start(outr_d.ap(), outr[:])

    nc.compile()
    return nc


# ======================= host-side prep =======================

def _out_perm():
    col = np.arange(NCEN)
    q = col // 128
    rr = col % 128
    t = rr // 16
    c16 = col % 16
    return 128 * t + 16 * q + c16


def prep_core_inputs(core, inp):
    f32 = np.float32
    b = core // 4
    kq = core % 4
    xyz = np.asarray(inp['xyz'], f32)[b]
    feat = np.asarray(inp['feature'], f32)[b]
    cen = xyz[1024 * kq:1024 * (kq + 1)]
    cfeat = feat[1024 * kq:1024 * (kq + 1)]

    d = {}
    d['cen4T'] = np.concatenate([-2.0 * cen.T, np.ones((1, NCEN), f32)], 0).astype(f32)
    d['pts4'] = np.concatenate([xyz.T, (xyz * xyz).sum(-1)[None, :]], 0).astype(f32)
    src = np.zeros((P, N), f32)
    for q in range(8):
        for r in range(3):
            src[16 * q + r] = xyz[:, r]
            src[16 * q + 3 + r] = feat[:, r]
    d['src16'] = src
    pf = np.arange(P)[:, None]
    ff = np.arange(256)[None, :]
    qq = ff // 32
    ii = 128 * (ff % 32) + pf
    cenidx = 128 * (ii // 512) + 16 * qq + (ii % 16)
    cpl = np.zeros((P, 6 * 256), f32)
    for r in range(3):
        cpl[:, 256 * r:256 * (r + 1)] = cen[cenidx, r]
        cpl[:, 256 * (3 + r):256 * (4 + r)] = cfeat[cenidx, r]
    d['cpl'] = cpl

    w0 = np.asarray(inp['conv0_w'], f32)
    w1 = np.asarray(inp['conv1_w'], f32)
    w2 = np.asarray(inp['conv2_w'], f32)

    # cst
    cst = np.zeros((P, CST_W), f32)
    cst[:, C_NEG1] = -1.0
    for g in range(8):
        cst[0:64, C_GW1 + g] = w1[8 * g:8 * g + 8].sum(0)
        cst[0:64, C_GW2 + g] = w2[16 * g:16 * g + 16].sum(0)
    cst[0:64, C_GI1:C_GI1 + 8] = (np.arange(64)[:, None] // 8 == np.arange(8)[None, :])
    cst[0:128, C_GI2:C_GI2 + 8] = (np.arange(128)[:, None] // 16 == np.arange(8)[None, :])
    cst[0:8, C_GE8:C_GE8 + 64] = (np.arange(64)[None, :] // 8 == np.arange(8)[:, None])
    cst[0:8, C_GE16:C_GE16 + 128] = (np.arange(128)[None, :] // 16 == np.arange(8)[:, None])
    A = np.zeros((196, 16), f32)
    for g in range(8):
        Qg = np.zeros((14, 14), f32)
        ug = np.zeros(14, f32)
        for c in range(8 * g, 8 * g + 8):
            Qg[:13, :13] += np.outer(w0[c], w0[c])
            ug[:13] += w0[c]
        A[:, g] = Qg.reshape(-1)
        Ug = np.zeros((14, 14), f32)
        Ug[:, 13] = ug
        A[:, 8 + g] = Ug.reshape(-1)
    cst[0:112, C_A0T:C_A0T + 16] = A[0:112]
    cst[0:84, C_A0T + 16:C_A0T + 32] = A[112:196]
    cst[0:64, C_G0] = np.asarray(inp['gn0_g'], f32).reshape(-1)
    cst[0:64, C_B0] = np.asarray(inp['gn0_b'], f32).reshape(-1)
    cst[0:64, C_G1] = np.asarray(inp['gn1_g'], f32).reshape(-1)
    cst[0:64, C_B1] = np.asarray(inp['gn1_b'], f32).reshape(-1)
    cst[0:128, C_G2] = np.asarray(inp['gn2_g'], f32).reshape(-1)
    cst[0:128, C_B2] = np.asarray(inp['gn2_b'], f32).reshape(-1)
    cst[0:2, C_BSELC] = np.array([1.0, 0.0], f32) if b == 0 else np.array([0.0, 1.0], f32)
    cst[0:16, C_BS0] = 1.0 if b == 0 else 0.0
    cst[0:16, C_BS1] = 1.0 if b == 1 else 0.0
    d['cst'] = cst

    # cstm
    cstm = np.zeros((P, CSTM_W), f32)
    cn = (cen * cen).sum(-1)
    cstm[:, M_THR:M_THR + 8] = R2 - cn.reshape(TILES, 128).T
    cstm[:, M_IOTA32:M_IOTA32 + 32] = np.arange(32, dtype=f32)[None, :]
    cstm[:, M_ID:M_ID + 64] = np.tile(np.eye(64, dtype=f32), (2, 1))
    cstm[:, M_WA] = np.asarray(inp['wa'], f32).reshape(-1)[0]
    cstm[:, M_WB] = np.asarray(inp['wb'], f32).reshape(-1)[0]
    cstm[:, M_WC] = np.asarray(inp['wc'], f32).reshape(-1)[0]
    cstm[:, M_ID128:M_ID128 + 128] = np.eye(128, dtype=f32)
    d['cstm'] = cstm

    # cstw (fp16)
    cstw = np.zeros((P, 256), np.float16)
    cstw[0:13, 0:64] = w0.T.astype(np.float16)
    cstw[0:64, 64:128] = w1.T.astype(np.float16)
    cstw[0:64, 128:256] = w2.T.astype(np.float16)
    d['cstw'] = cstw
    return d


_NC_CACHE = {}


def kernel(**inputs):
    if 'nc' not in _NC_CACHE:
        _NC_CACHE['nc'] = build_program()
    nc = _NC_CACHE['nc']
    in_maps = [prep_core_inputs(c, inputs) for c in range(8)]
    res = bass_utils.run_bass_kernel_spmd(nc, in_maps, core_ids=list(range(8)))
    perm = _out_perm()
    out_r = np.zeros((B, 128, N), np.float32)
    out_x = np.zeros((B, 128, N), np.float32)
    for c in range(8):
        b = c // 4
        kq = c % 4
        out_x[b][:, 1024 * kq + perm] = res.results[c]['outx']
        out_r[b][:, 1024 * kq + perm] = res.results[c]['outr']
    return (out_r, out_x)
